# revision 12
# baseline (speedup 1.0000x reference)
"""BiLSTM tagger kernel for 8 Trainium2 NeuronCores.

Strategy: data-parallel over batch (16 sequences per core, weights
replicated). Per core, the two directions of each BiLSTM layer run as
col-group-packed scans: scan0 lives entirely at SBUF/PSUM partitions 0:16
(PE column group 0), scan1 at partitions 32:48 (column group 1), so their
recurrent matmuls execute CONCURRENTLY on different 32-column strips of
the 128x128 PE array (tile_position col tiling). All matmuls run in bf16
(fp32 matmul is 4x slower on TRN2); PSUM accumulation stays fp32.

Recurrent step layout: stationary = h^T chunks [128,16], moving = W_hh^T
slices, psum gates live in two [48, 1024] tiles (A: i,f / B: o,g) whose
partition strips 0:16 / 32:48 hold scan0 / scan1 — the strips share PSUM
banks; has_written tracking is per-partition so the per-strip
inject(start=True) + accumulate(start=False) groups are independent
(validated on HW). gx (input projections, precomputed per layer into HBM)
is injected into PSUM via identity matmuls during the previous step's
tail. h is re-transposed each step with four tiny matmuls against a 16x16
identity per scan (row groups 0 / 1, also concurrent). Backward scans
consume inputs pre-reversed per sequence length (host permutation indices
+ indirect DMA); outputs stay in scan order and are un-reversed by the
consumers' row gathers. Steps past a sequence's length compute garbage
that is masked to zero and cannot contaminate earlier steps.
"""

import sys

for _p in ("/opt/trn_rl_repo",):
    if _p not in sys.path:
        sys.path.append(_p)

import numpy as np
import ml_dtypes

import concourse.bass as bass
import concourse.tile as tile
from concourse import bacc, mybir
from concourse.bass import IndirectOffsetOnAxis
from concourse.bass_utils import run_bass_kernel_spmd

F32 = mybir.dt.float32
BF16 = mybir.dt.bfloat16
I32 = mybir.dt.int32
AF = mybir.ActivationFunctionType
ALU = mybir.AluOpType

# problem sizes (full / per-core)
B, T, V, E, H, TAGS = 128, 512, 50000, 256, 512, 64
NC = 8
BL = B // NC   # 16 sequences per core
G = 4 * H      # 2048 gate width

# permutation taking pytorch gate order i,f,g,o -> i,f,o,g (sigmoid block first)
_GATE_PERM = np.concatenate([
    np.arange(0, H), np.arange(H, 2 * H), np.arange(3 * H, 4 * H),
    np.arange(2 * H, 3 * H)])

# partition strip (= PE column group offset) per scan slot
_STRIP = (0, 32)


def _build(nc, Tn=T, Bl=BL, TC=2, RC=4, ablate=()):
    """Emit the per-core program. Tn shrinkable for dev testing.
    ablate: subset of {"scan","proj","cls"} to skip (timing attribution)."""
    ntok = Bl * Tn
    nchunk = ntok // 128
    KE = E // 128       # k-chunks for layer-1 input proj
    KH2 = 2 * H // 128  # k-chunks for layer-2 input proj / classifier
    KH = H // 128       # k-chunks for recurrent
    assert ntok % 128 == 0

    # ---- dram I/O ----
    emb = nc.dram_tensor("emb", [V, E], F32, kind="ExternalInput")
    xf_idx = nc.dram_tensor("xf_idx", [128, nchunk], I32, kind="ExternalInput")
    xb_idx = nc.dram_tensor("xb_idx", [128, nchunk], I32, kind="ExternalInput")
    rev128 = nc.dram_tensor("rev128", [128, nchunk], I32, kind="ExternalInput")
    mask = nc.dram_tensor("mask", [Bl, Tn], F32, kind="ExternalInput")
    ident = nc.dram_tensor("ident", [16, 16], BF16, kind="ExternalInput")

    wih, whh, biasd = {}, {}, {}
    for s, din in (("f1", E), ("b1", E), ("f2", 2 * H), ("b2", 2 * H)):
        wih[s] = nc.dram_tensor(f"wihT_{s}", [din, G], BF16, kind="ExternalInput")
        whh[s] = nc.dram_tensor(f"whhT_{s}", [H, G], BF16, kind="ExternalInput")
        biasd[s] = nc.dram_tensor(f"bias_{s}", [128, G], F32, kind="ExternalInput")
    wcls = nc.dram_tensor("wclsT", [2 * H, TAGS], BF16, kind="ExternalInput")
    bcls = nc.dram_tensor("bcls", [TAGS, 1], F32, kind="ExternalInput")

    gx = {s: nc.dram_tensor(f"gx_{s}", [ntok, G], BF16)
          for s in ("f1", "b1", "f2", "b2")}
    # per-direction layer outputs; backward halves stay in scan order and are
    # un-reversed by the consumers' row gathers (no per-step scatters)
    hout = {s: nc.dram_tensor(f"hout_{s}", [ntok, H], BF16)
            for s in ("f1", "b1", "f2", "b2")}
    logitsT = nc.dram_tensor("logitsT", [TAGS, ntok], F32, kind="ExternalOutput")

    with tile.TileContext(nc) as tc:
        with tc.tile_pool(name="const", bufs=1) as cpool:
            def load_const(nm, shape, dt, src_ap):
                t = cpool.tile(shape, dt, name=nm, tag=nm)
                nc.gpsimd.dma_start(t[:], src_ap)
                return t

            xf_sb = load_const("xf_sb", [128, nchunk], I32, xf_idx[:])
            xb_sb = load_const("xb_sb", [128, nchunk], I32, xb_idx[:])
            rev128_sb = load_const("rev128_sb", [128, nchunk], I32, rev128[:])
            bcls_sb = load_const("bcls_sb", [TAGS, 1], F32, bcls[:])
            bias_sb = {s: load_const(f"bias_sb_{s}", [128, G], F32, biasd[s][:])
                       for s in ("f1", "b1", "f2", "b2")}
            # identity + mask replicated into both scan strips
            id_all = cpool.tile([48, 16], BF16, name="id_all")
            mask_all = cpool.tile([48, Tn], F32, name="mask_all")
            for p in _STRIP:
                nc.gpsimd.dma_start(id_all[p:p + 16, :], ident[:])
                nc.gpsimd.dma_start(mask_all[p:p + 16, :], mask[:])
            wcls_sb = cpool.tile([128, KH2, TAGS], BF16, name="wcls_sb")
            for k in range(KH2):
                nc.gpsimd.dma_start(wcls_sb[:, k, :], wcls[128 * k:128 * (k + 1), :])

            # layer-1 input projections (inputs gathered from embedding table)
            if "proj" not in ablate:
                _proj_phase(nc, tc, nchunk, KE, wih=wih, bias_sb=bias_sb, gx=gx,
                            jobs=[("f1", emb, xf_sb, True), ("b1", emb, xb_sb, True)])
            # layer-1 scans
            if "scan" not in ablate:
                _scan_phase(nc, tc, Tn, Bl, TC, RC, KH,
                            scans=("f1", "b1"), whh=whh, gx=gx, hout=hout,
                            mask_all=mask_all, id_all=id_all)
            # layer-2 input projections: input token (b,t) for the fwd scan is
            # [f1h[t], s1h[rev(t)]]; for the bwd scan it is [f1h[rev(t)], s1h[t]]
            if "proj" not in ablate:
                _proj_phase(nc, tc, nchunk, KH2, wih=wih, bias_sb=bias_sb, gx=gx,
                            jobs=[("f2", (hout["f1"], None, hout["b1"], rev128_sb), None, False),
                                  ("b2", (hout["f1"], rev128_sb, hout["b1"], None), None, False)])
            # layer-2 scans
            if "scan" not in ablate:
                _scan_phase(nc, tc, Tn, Bl, TC, RC, KH,
                            scans=("f2", "b2"), whh=whh, gx=gx, hout=hout,
                            mask_all=mask_all, id_all=id_all)

            if "cls" in ablate:
                lg0 = cpool.tile([TAGS, 128], F32, name="lg0")
                nc.vector.memset(lg0[:], 0.0)
                for c in range(nchunk):
                    nc.gpsimd.dma_start(logitsT[:, 128 * c:128 * (c + 1)], lg0[:])
                return nc
            # classifier: logits^T = W_cls @ out2^T + b_cls
            with tc.tile_pool(name="cls", bufs=3) as gp, \
                 tc.tile_pool(name="clsT", bufs=3) as gtp, \
                 tc.tile_pool(name="clsps", bufs=4, space="PSUM") as pp, \
                 tc.tile_pool(name="clso", bufs=3) as op:
                for c in range(nchunk):
                    o2 = gp.tile([128, 2 * H], BF16, tag="in")
                    nc.gpsimd.dma_start(o2[:, 0:H], hout["f2"][128 * c:128 * (c + 1), :])
                    nc.gpsimd.indirect_dma_start(
                        out=o2[:, H:2 * H], out_offset=None, in_=hout["b2"][:],
                        in_offset=IndirectOffsetOnAxis(ap=rev128_sb[:, c:c + 1], axis=0))
                    o2T = gtp.tile([128, KH2, 128], BF16, tag="inT")
                    for k in range(KH2):
                        nc.sync.dma_start_transpose(
                            o2T[:, k, :], o2[:, 128 * k:128 * (k + 1)])
                    ps = pp.tile([TAGS, 128], F32, name="clsps_t")
                    for k in range(KH2):
                        nc.tensor.matmul(ps[:], wcls_sb[:, k, :], o2T[:, k, :],
                                         start=(k == 0), stop=(k == KH2 - 1))
                    lg = op.tile([TAGS, 128], F32, tag="lg")
                    nc.scalar.activation(lg[:], ps[:], AF.Identity,
                                         bias=bcls_sb[:, 0:1])
                    nc.gpsimd.dma_start(logitsT[:, 128 * c:128 * (c + 1)], lg[:])
    return nc


def _proj_phase(nc, tc, nchunk, KD, wih, bias_sb, gx, jobs):
    """gx_s = input @ W_ih_s^T + b_s, written contiguously in scan-time order.

    jobs: (scan_name, dram_src, idx_tile_or_None, is_emb). For is_emb the idx
    tile holds embedding row ids (fp32 gather + cast); otherwise rows of src
    are read contiguously (idx None) or gathered (idx set, layer-2 backward).
    """
    D = KD * 128
    with tc.tile_pool(name="pw", bufs=1) as wpool, \
         tc.tile_pool(name="pg", bufs=3) as gpool, \
         tc.tile_pool(name="pgT", bufs=3) as tpool, \
         tc.tile_pool(name="pps", bufs=4, space="PSUM") as ppool, \
         tc.tile_pool(name="pout", bufs=3) as opool:
        wsb = {}
        for s, _, _, _ in jobs:
            wsb[s] = wpool.tile([128, KD, G], BF16, tag=f"w{s}", name=f"wih_{s}")
            for k in range(KD):
                nc.gpsimd.dma_start(wsb[s][:, k, :], wih[s][128 * k:128 * (k + 1), :])
        for c in range(nchunk):
            for s, dsrc, idx, is_emb in jobs:
                if is_emb:
                    e32 = gpool.tile([128, D], F32, tag="e32")
                    nc.gpsimd.indirect_dma_start(
                        out=e32[:], out_offset=None, in_=dsrc[:],
                        in_offset=IndirectOffsetOnAxis(ap=idx[:, c:c + 1], axis=0))
                    xin = gpool.tile([128, D], BF16, tag="e16")
                    nc.vector.tensor_copy(xin[:], e32[:])
                else:
                    fsrc, fidx, bsrc, bidx = dsrc
                    xin = gpool.tile([128, D], BF16, tag="e16")
                    for src_t, sidx, lo in ((fsrc, fidx, 0), (bsrc, bidx, H)):
                        if sidx is None:
                            nc.gpsimd.dma_start(xin[:, lo:lo + H],
                                                src_t[128 * c:128 * (c + 1), :])
                        else:
                            nc.gpsimd.indirect_dma_start(
                                out=xin[:, lo:lo + H], out_offset=None, in_=src_t[:],
                                in_offset=IndirectOffsetOnAxis(ap=sidx[:, c:c + 1], axis=0))
                xT = tpool.tile([128, KD, 128], BF16, tag="xT")
                for k in range(KD):
                    nc.sync.dma_start_transpose(
                        xT[:, k, :], xin[:, 128 * k:128 * (k + 1)])
                gout = opool.tile([128, G], BF16, tag="gout")
                for n in range(G // 512):
                    ps = ppool.tile([128, 512], F32, name="pps")
                    for k in range(KD):
                        nc.tensor.matmul(
                            ps[:], xT[:, k, :], wsb[s][:, k, 512 * n:512 * (n + 1)],
                            start=(k == 0), stop=(k == KD - 1))
                    nc.vector.tensor_tensor(
                        out=gout[:, 512 * n:512 * (n + 1)], in0=ps[:],
                        in1=bias_sb[s][:, 512 * n:512 * (n + 1)],
                        op=ALU.add)
                nc.gpsimd.dma_start(gx[s][128 * c:128 * (c + 1), :], gout[:])


def _scan_phase(nc, tc, Tn, Bl, TC, RC, KH, scans, whh, gx, hout,
                mask_all, id_all):
    """Col-group packed scans: scan i occupies partition strip _STRIP[i]
    (PE column group i). Software-pipelined gx injection: next step's gx
    lands in PSUM via identity matmuls during this step's idle PE window;
    recurrent matmuls then accumulate onto it (start=False) and ScalarE
    reads gates straight from PSUM. Gates live in two 2-bank [48, 1024]
    tiles (A: i,f / B: o,g) whose partition strips are per-scan."""
    gxv = {s: gx[s].ap().rearrange("(b t) d -> b t d", b=Bl) for s in scans}
    houtv = {s: hout[s].ap().rearrange("(b t) d -> b t d", b=Bl) for s in scans}
    H2 = 2 * H
    strip = {s: _STRIP[i] for i, s in enumerate(scans)}

    def sl(s):
        return slice(strip[s], strip[s] + 16)

    with tc.tile_pool(name="sw", bufs=1) as wpool, \
         tc.tile_pool(name="sgx", bufs=4) as gxpool, \
         tc.tile_pool(name="sst", bufs=1) as stpool, \
         tc.tile_pool(name="sps", bufs=4, space="PSUM") as pspool, \
         tc.tile_pool(name="swk", bufs=3) as wkpool, \
         tc.tile_pool(name="shT", bufs=3) as htpool, \
         tc.tile_pool(name="srng", bufs=3) as rpool:
        wsb, hT = {}, {}
        for s in scans:
            wsb[s] = wpool.tile([128, KH, G], BF16, tag=f"whh{s}", name=f"whh_{s}")
            for k in range(KH):
                nc.gpsimd.dma_start(wsb[s][:, k, :], whh[s][128 * k:128 * (k + 1), :])
            hT[s] = htpool.tile([128, KH * Bl], BF16, tag="hT", name="hT0")
            nc.vector.memset(hT[s][:], 0.0)
        c_all = stpool.tile([48, H], F32, tag="c", name="c_all")
        nc.vector.memset(c_all[:], 0.0)
        gxc = {}
        gA = [None]
        gB = [None]
        ring = [None]
        nwin = (Tn + TC - 1) // TC

        def load_gx(w):
            # prefetch gx window w (steps w*TC .. w*TC+TC-1)
            tl = gxpool.tile([48, TC, G], BF16, tag="gx", name="gxc")
            for s in scans:
                nc.gpsimd.dma_start(tl[sl(s), :, :],
                                    gxv[s][:, w * TC:(w + 1) * TC, :])
            gxc[w] = tl
            gxc.pop(w - 3, None)

        def inject(tt, only=None):
            # psum halves for step tt, pre-filled with gx via identity matmuls
            if only is None or only == scans[0]:
                gA[0] = pspool.tile([48, H2], F32, tag="ps", name="gA")
                gB[0] = pspool.tile([48, H2], F32, tag="ps", name="gB")
            gxt = gxc[tt // TC]
            for s in (scans if only is None else [only]):
                p = strip[s]
                for half, lo in ((gA[0], 0), (gB[0], H2)):
                    for n in range(2):
                        nc.tensor.matmul(
                            half[sl(s), 512 * n:512 * (n + 1)], id_all[sl(s), :],
                            gxt[sl(s), tt % TC, lo + 512 * n:lo + 512 * (n + 1)],
                            start=True, stop=False, skip_group_check=True,
                            tile_position=(p, p))

        load_gx(0)
        if nwin > 1:
            load_gx(1)
        inject(0)
        for t in range(Tn):
            # recurrent matmuls accumulate onto the injected gx; emitted as
            # one burst per scan so scan f's gate math (ScalarE/VectorE)
            # overlaps scan b's matmul burst on the PE
            gAc, gBc = gA[0], gB[0]
            for s in scans:
                for half, cols in ((gAc, (0, 1)), (gBc, (3, 2))):
                    for n in cols:
                        dst_lo = 512 * (n % 2)
                        for k in range(KH):
                            nc.tensor.matmul(
                                half[sl(s), dst_lo:dst_lo + 512],
                                hT[s][:, Bl * k:Bl * (k + 1)],
                                wsb[s][:, k, 512 * n:512 * (n + 1)],
                                start=False, stop=(k == KH - 1),
                                skip_group_check=True,
                                tile_position=(0, strip[s]))
            gact = wkpool.tile([48, G], F32, tag="gact", name="gact")
            if t % RC == 0:
                ring[0] = rpool.tile([48, RC, H], BF16, tag="ring", name="ring")
            for s in scans:
                nc.scalar.activation(gact[sl(s), 0:H2], gAc[sl(s), :], AF.Sigmoid)
            for s in scans:
                nc.scalar.activation(gact[sl(s), 3 * H:G], gBc[sl(s), H:H2], AF.Tanh)
            for s in scans:
                nc.scalar.activation(gact[sl(s), H2:3 * H], gBc[sl(s), 0:H], AF.Sigmoid)
            # per-scan elementwise chains on different engines: scan0 on the
            # vector engine, scan1 on gpsimd (all-SBUF operands) — halves
            # queue contention; activations stay on ScalarE
            eng = {scans[0]: nc.vector, scans[1]: nc.gpsimd}
            t1 = wkpool.tile([48, H], F32, tag="t1", name="t1")
            t2 = wkpool.tile([48, H], F32, tag="t2", name="t2")
            for s in scans:
                eng[s].tensor_tensor(out=t1[sl(s), :], in0=gact[sl(s), H:H2],
                                     in1=c_all[sl(s), :], op=ALU.mult)
            for s in scans:
                eng[s].tensor_tensor(out=t2[sl(s), :], in0=gact[sl(s), 0:H],
                                     in1=gact[sl(s), 3 * H:G], op=ALU.mult)
            for s in scans:
                eng[s].tensor_tensor(out=c_all[sl(s), :], in0=t1[sl(s), :],
                                     in1=t2[sl(s), :], op=ALU.add)
            tch = wkpool.tile([48, H], F32, tag="tch", name="tch")
            for s in scans:
                nc.scalar.activation(tch[sl(s), :], c_all[sl(s), :], AF.Tanh)
            h16 = wkpool.tile([48, H], BF16, tag="h16", name="h16")
            for s in scans:
                eng[s].tensor_tensor(out=h16[sl(s), :], in0=gact[sl(s), H2:3 * H],
                                     in1=tch[sl(s), :], op=ALU.mult)
            # allocate transpose psum tiles first (keeps the slot rotation),
            # then emit next step's gx injects BEFORE the transpose matmuls
            # so they fill the PE window spent waiting for h16
            hT_ps = {}
            for s in scans:
                hT_ps[s] = pspool.tile([128, KH * Bl], F32, tag="ps", name="hT_ps")
            if t + 1 < Tn:
                if (t + 1) % TC == 0 and (t + 1) // TC + 1 < nwin:
                    load_gx((t + 1) // TC + 1)
                inject(t + 1, only=scans[0])
            for s in scans:
                for k in range(KH):
                    nc.tensor.matmul(hT_ps[s][:, Bl * k:Bl * (k + 1)],
                                     h16[sl(s), 128 * k:128 * (k + 1)],
                                     id_all[sl(s), :], start=True, stop=True,
                                     tile_position=(strip[s], 0))
                hTn = htpool.tile([128, KH * Bl], BF16, tag="hT", name="hTn")
                nc.vector.tensor_copy(hTn[:], hT_ps[s][:])
                hT[s] = hTn
            if t + 1 < Tn:
                inject(t + 1, only=scans[1])
            for s in scans:
                eng[s].tensor_scalar_mul(ring[0][sl(s), t % RC, :], h16[sl(s), :],
                                         mask_all[sl(s), t:t + 1])
            if (t + 1) % RC == 0:
                t0r = t + 1 - RC
                for s in scans:
                    nc.gpsimd.dma_start(houtv[s][:, t0r:t0r + RC, :],
                                        ring[0][sl(s), :, :])


def _prep_inputs(inputs, Tn=T, Bl=BL, ncores=NC):
    """Host-side sharding + weight preprocessing. Returns per-core in_maps."""
    x = np.asarray(inputs["x"]).astype(np.int32)
    lengths = np.asarray(inputs["lengths"]).astype(np.int32)
    emb = np.asarray(inputs["emb"], dtype=np.float32)
    ntok = Bl * Tn

    com = {"emb": emb, "ident": np.eye(16, dtype=ml_dtypes.bfloat16)}
    for s in ("f1", "b1", "f2", "b2"):
        w_ih = np.asarray(inputs[f"W_ih_{s}"], np.float32)[_GATE_PERM]
        w_hh = np.asarray(inputs[f"W_hh_{s}"], np.float32)[_GATE_PERM]
        b = np.asarray(inputs[f"b_{s}"], np.float32)[_GATE_PERM]
        com[f"wihT_{s}"] = np.ascontiguousarray(w_ih.T).astype(ml_dtypes.bfloat16)
        com[f"whhT_{s}"] = np.ascontiguousarray(w_hh.T).astype(ml_dtypes.bfloat16)
        com[f"bias_{s}"] = np.tile(b.reshape(1, G), (128, 1))
    com["wclsT"] = np.ascontiguousarray(
        np.asarray(inputs["W_cls"], np.float32).T).astype(ml_dtypes.bfloat16)
    com["bcls"] = np.asarray(inputs["b_cls"], np.float32).reshape(TAGS, 1)

    def chunked(a):  # [ntok] -> [128, ntok//128] with chunk c in column c
        return np.ascontiguousarray(a.reshape(-1).reshape(ntok // 128, 128).T)

    in_maps = []
    for c in range(ncores):
        xs = x[Bl * c:Bl * (c + 1), :Tn]
        ls = np.minimum(lengths[Bl * c:Bl * (c + 1)], Tn)
        ts = np.arange(Tn)[None, :]
        rev = np.where(ts < ls[:, None], ls[:, None] - 1 - ts, ts)  # [Bl,Tn]
        xrev = np.take_along_axis(xs, rev, axis=1)
        flat_rev = (np.arange(Bl)[:, None] * Tn + rev).astype(np.int32)
        m = {
            "xf_idx": chunked(xs),
            "xb_idx": chunked(xrev),
            "rev128": chunked(flat_rev),
            "mask": (ts < ls[:, None]).astype(np.float32),
        }
        m.update(com)
        in_maps.append(m)
    return in_maps


_CACHED = {}


def kernel(**inputs) -> np.ndarray:
    if "nc" not in _CACHED:
        nc = bacc.Bacc("TRN2", target_bir_lowering=False, debug=False,
                       num_devices=NC)
        _build(nc)
        nc.compile()
        _CACHED["nc"] = nc
    nc = _CACHED["nc"]
    in_maps = _prep_inputs(inputs)
    res = run_bass_kernel_spmd(nc, in_maps, core_ids=list(range(NC)), trace=False)
    outs = []
    for c in range(NC):
        lt = res.results[c]["logitsT"]  # [TAGS, ntok]
        outs.append(np.ascontiguousarray(lt.T.reshape(BL, T, TAGS)))
    return np.concatenate(outs, axis=0).astype(np.float32)


# revision 18
# speedup vs baseline: 1.1658x; 1.1658x over previous
"""BiLSTM tagger kernel for 8 Trainium2 NeuronCores.

Strategy: data-parallel over batch (16 sequences per core, weights
replicated). Per core, the two directions of each BiLSTM layer run as
col-group-packed scans: scan0 lives entirely at SBUF/PSUM partitions 0:16
(PE column group 0), scan1 at partitions 32:48 (column group 1), so their
recurrent matmuls execute CONCURRENTLY on different 32-column strips of
the 128x128 PE array (tile_position col tiling). All matmuls run in bf16
(fp32 matmul is 4x slower on TRN2); PSUM accumulation stays fp32.

Recurrent step layout: stationary = h^T chunks [128,16], moving = W_hh^T
slices, psum gates live in two [48, 1024] tiles (A: i,f / B: o,g) whose
partition strips 0:16 / 32:48 hold scan0 / scan1 — the strips share PSUM
banks; has_written tracking is per-partition so the per-strip
inject(start=True) + accumulate(start=False) groups are independent
(validated on HW). gx (input projections, precomputed per layer into HBM)
is injected into PSUM via identity matmuls during the previous step's
tail. h is re-transposed each step with four tiny matmuls against a 16x16
identity per scan (row groups 0 / 1, also concurrent). Backward scans
consume inputs pre-reversed per sequence length (host permutation indices
+ indirect DMA); outputs stay in scan order and are un-reversed by the
consumers' row gathers. Steps past a sequence's length compute garbage
that is masked to zero and cannot contaminate earlier steps.
"""

import sys

for _p in ("/opt/trn_rl_repo",):
    if _p not in sys.path:
        sys.path.append(_p)

import numpy as np
import ml_dtypes

import concourse.bass as bass
import concourse.tile as tile
from concourse import bacc, mybir
from concourse.bass import IndirectOffsetOnAxis
from concourse.bass_utils import run_bass_kernel_spmd

F32 = mybir.dt.float32
BF16 = mybir.dt.bfloat16
I32 = mybir.dt.int32
AF = mybir.ActivationFunctionType
ALU = mybir.AluOpType

# problem sizes (full / per-core)
B, T, V, E, H, TAGS = 128, 512, 50000, 256, 512, 64
NC = 8
BL = B // NC   # 16 sequences per core
G = 4 * H      # 2048 gate width

# permutation taking pytorch gate order i,f,g,o -> i,f,o,g (sigmoid block first)
_GATE_PERM = np.concatenate([
    np.arange(0, H), np.arange(H, 2 * H), np.arange(3 * H, 4 * H),
    np.arange(2 * H, 3 * H)])

# partition strip (= PE column group offset) per scan slot
_STRIP = (0, 32)


def _build(nc, Tn=T, Bl=BL, TC=2, RC=4, ablate=()):
    """Emit the per-core program. Tn shrinkable for dev testing.
    ablate: subset of {"scan","proj","cls"} to skip (timing attribution)."""
    ntok = Bl * Tn
    nchunk = ntok // 128
    KE = E // 128       # k-chunks for layer-1 input proj
    KH2 = 2 * H // 128  # k-chunks for layer-2 input proj / classifier
    KH = H // 128       # k-chunks for recurrent
    assert ntok % 128 == 0

    # ---- dram I/O ----
    emb = nc.dram_tensor("emb", [V, E], F32, kind="ExternalInput")
    xf_idx = nc.dram_tensor("xf_idx", [128, nchunk], I32, kind="ExternalInput")
    xb_idx = nc.dram_tensor("xb_idx", [128, nchunk], I32, kind="ExternalInput")
    rev128 = nc.dram_tensor("rev128", [128, nchunk], I32, kind="ExternalInput")
    mask = nc.dram_tensor("mask", [Bl, Tn], F32, kind="ExternalInput")
    ident = nc.dram_tensor("ident", [16, 16], BF16, kind="ExternalInput")

    wih, whh, biasd = {}, {}, {}
    for s, din in (("f1", E), ("b1", E), ("f2", 2 * H), ("b2", 2 * H)):
        wih[s] = nc.dram_tensor(f"wihT_{s}", [din, G], BF16, kind="ExternalInput")
        whh[s] = nc.dram_tensor(f"whhT_{s}", [H, G], BF16, kind="ExternalInput")
        biasd[s] = nc.dram_tensor(f"bias_{s}", [128, G], F32, kind="ExternalInput")
    wcls = nc.dram_tensor("wclsT", [2 * H, TAGS], BF16, kind="ExternalInput")
    bcls = nc.dram_tensor("bcls", [TAGS, 1], F32, kind="ExternalInput")

    gx = {s: nc.dram_tensor(f"gx_{s}", [ntok, G], BF16)
          for s in ("f1", "b1", "f2", "b2")}
    # per-direction layer outputs; backward halves stay in scan order and are
    # un-reversed by the consumers' row gathers (no per-step scatters)
    hout = {s: nc.dram_tensor(f"hout_{s}", [ntok, H], BF16)
            for s in ("f1", "b1", "f2", "b2")}
    logitsT = nc.dram_tensor("logitsT", [TAGS, ntok], F32, kind="ExternalOutput")

    with tile.TileContext(nc) as tc:
        with tc.tile_pool(name="const", bufs=1) as cpool:
            def load_const(nm, shape, dt, src_ap):
                t = cpool.tile(shape, dt, name=nm, tag=nm)
                nc.gpsimd.dma_start(t[:], src_ap)
                return t

            xf_sb = load_const("xf_sb", [128, nchunk], I32, xf_idx[:])
            xb_sb = load_const("xb_sb", [128, nchunk], I32, xb_idx[:])
            rev128_sb = load_const("rev128_sb", [128, nchunk], I32, rev128[:])
            bcls_sb = load_const("bcls_sb", [TAGS, 1], F32, bcls[:])
            bias_sb = {s: load_const(f"bias_sb_{s}", [128, G], F32, biasd[s][:])
                       for s in ("f1", "b1", "f2", "b2")}
            # identity + mask replicated into both scan strips
            id_all = cpool.tile([48, 16], BF16, name="id_all")
            mask_all = cpool.tile([48, Tn], F32, name="mask_all")
            for p in _STRIP:
                nc.gpsimd.dma_start(id_all[p:p + 16, :], ident[:])
                nc.gpsimd.dma_start(mask_all[p:p + 16, :], mask[:])
            wcls_sb = cpool.tile([128, KH2, TAGS], BF16, name="wcls_sb")
            for k in range(KH2):
                nc.gpsimd.dma_start(wcls_sb[:, k, :], wcls[128 * k:128 * (k + 1), :])

            # layer-1 input projections (inputs gathered from embedding table)
            if "proj" not in ablate:
                _proj_phase(nc, tc, nchunk, KE, wih=wih, bias_sb=bias_sb, gx=gx,
                            jobs=[("f1", emb, xf_sb, True), ("b1", emb, xb_sb, True)])
            # layer-1 scans
            if "scan" not in ablate:
                _scan_phase(nc, tc, Tn, Bl, TC, RC, KH,
                            scans=("f1", "b1"), whh=whh, gx=gx, hout=hout,
                            mask_all=mask_all, id_all=id_all)
            # layer-2 input projections: input token (b,t) for the fwd scan is
            # [f1h[t], s1h[rev(t)]]; for the bwd scan it is [f1h[rev(t)], s1h[t]]
            if "proj" not in ablate:
                _proj_phase(nc, tc, nchunk, KH2, wih=wih, bias_sb=bias_sb, gx=gx,
                            jobs=[("f2", (hout["f1"], None, hout["b1"], rev128_sb), None, False),
                                  ("b2", (hout["f1"], rev128_sb, hout["b1"], None), None, False)])
            # layer-2 scans
            if "scan" not in ablate:
                _scan_phase(nc, tc, Tn, Bl, TC, RC, KH,
                            scans=("f2", "b2"), whh=whh, gx=gx, hout=hout,
                            mask_all=mask_all, id_all=id_all)

            if "cls" in ablate:
                lg0 = cpool.tile([TAGS, 128], F32, name="lg0")
                nc.vector.memset(lg0[:], 0.0)
                for c in range(nchunk):
                    nc.gpsimd.dma_start(logitsT[:, 128 * c:128 * (c + 1)], lg0[:])
                return nc
            # classifier: logits^T = W_cls @ out2^T + b_cls
            with tc.tile_pool(name="cls", bufs=3) as gp, \
                 tc.tile_pool(name="clsT", bufs=3) as gtp, \
                 tc.tile_pool(name="clsps", bufs=4, space="PSUM") as pp, \
                 tc.tile_pool(name="clso", bufs=3) as op:
                for c in range(nchunk):
                    o2 = gp.tile([128, 2 * H], BF16, tag="in")
                    nc.gpsimd.dma_start(o2[:, 0:H], hout["f2"][128 * c:128 * (c + 1), :])
                    nc.gpsimd.indirect_dma_start(
                        out=o2[:, H:2 * H], out_offset=None, in_=hout["b2"][:],
                        in_offset=IndirectOffsetOnAxis(ap=rev128_sb[:, c:c + 1], axis=0))
                    o2T = gtp.tile([128, KH2, 128], BF16, tag="inT")
                    for k in range(KH2):
                        nc.sync.dma_start_transpose(
                            o2T[:, k, :], o2[:, 128 * k:128 * (k + 1)])
                    ps = pp.tile([TAGS, 128], F32, name="clsps_t")
                    for k in range(KH2):
                        nc.tensor.matmul(ps[:], wcls_sb[:, k, :], o2T[:, k, :],
                                         start=(k == 0), stop=(k == KH2 - 1))
                    lg = op.tile([TAGS, 128], F32, tag="lg")
                    nc.scalar.activation(lg[:], ps[:], AF.Identity,
                                         bias=bcls_sb[:, 0:1])
                    nc.gpsimd.dma_start(logitsT[:, 128 * c:128 * (c + 1)], lg[:])
    return nc


def _proj_phase(nc, tc, nchunk, KD, wih, bias_sb, gx, jobs):
    """gx_s = input @ W_ih_s^T + b_s, written contiguously in scan-time order.

    jobs: (scan_name, dram_src, idx_tile_or_None, is_emb). For is_emb the idx
    tile holds embedding row ids (fp32 gather + cast); otherwise rows of src
    are read contiguously (idx None) or gathered (idx set, layer-2 backward).
    """
    D = KD * 128
    with tc.tile_pool(name="pw", bufs=1) as wpool, \
         tc.tile_pool(name="pg", bufs=3) as gpool, \
         tc.tile_pool(name="pgT", bufs=3) as tpool, \
         tc.tile_pool(name="pps", bufs=4, space="PSUM") as ppool, \
         tc.tile_pool(name="pout", bufs=3) as opool:
        wsb = {}
        for s, _, _, _ in jobs:
            wsb[s] = wpool.tile([128, KD, G], BF16, tag=f"w{s}", name=f"wih_{s}")
            for k in range(KD):
                nc.gpsimd.dma_start(wsb[s][:, k, :], wih[s][128 * k:128 * (k + 1), :])
        for c in range(nchunk):
            for s, dsrc, idx, is_emb in jobs:
                if is_emb:
                    e32 = gpool.tile([128, D], F32, tag="e32")
                    nc.gpsimd.indirect_dma_start(
                        out=e32[:], out_offset=None, in_=dsrc[:],
                        in_offset=IndirectOffsetOnAxis(ap=idx[:, c:c + 1], axis=0))
                    xin = gpool.tile([128, D], BF16, tag="e16")
                    nc.vector.tensor_copy(xin[:], e32[:])
                else:
                    fsrc, fidx, bsrc, bidx = dsrc
                    xin = gpool.tile([128, D], BF16, tag="e16")
                    for src_t, sidx, lo in ((fsrc, fidx, 0), (bsrc, bidx, H)):
                        if sidx is None:
                            nc.gpsimd.dma_start(xin[:, lo:lo + H],
                                                src_t[128 * c:128 * (c + 1), :])
                        else:
                            nc.gpsimd.indirect_dma_start(
                                out=xin[:, lo:lo + H], out_offset=None, in_=src_t[:],
                                in_offset=IndirectOffsetOnAxis(ap=sidx[:, c:c + 1], axis=0))
                xT = tpool.tile([128, KD, 128], BF16, tag="xT")
                for k in range(KD):
                    nc.sync.dma_start_transpose(
                        xT[:, k, :], xin[:, 128 * k:128 * (k + 1)])
                gout = opool.tile([128, G], BF16, tag="gout")
                for n in range(G // 512):
                    ps = ppool.tile([128, 512], F32, name="pps")
                    for k in range(KD):
                        nc.tensor.matmul(
                            ps[:], xT[:, k, :], wsb[s][:, k, 512 * n:512 * (n + 1)],
                            start=(k == 0), stop=(k == KD - 1))
                    nc.vector.tensor_tensor(
                        out=gout[:, 512 * n:512 * (n + 1)], in0=ps[:],
                        in1=bias_sb[s][:, 512 * n:512 * (n + 1)],
                        op=ALU.add)
                nc.gpsimd.dma_start(gx[s][128 * c:128 * (c + 1), :], gout[:])


def _scan_phase(nc, tc, Tn, Bl, TC, RC, KH, scans, whh, gx, hout,
                mask_all, id_all):
    """Col-group packed scans: scan i occupies partition strip _STRIP[i]
    (PE column group i). Software-pipelined gx injection: next step's gx
    lands in PSUM via identity matmuls during this step's idle PE window;
    recurrent matmuls then accumulate onto it (start=False) and ScalarE
    reads gates straight from PSUM. Gates live in two 2-bank [48, 1024]
    tiles (A: i,f / B: o,g) whose partition strips are per-scan."""
    gxv = {s: gx[s].ap().rearrange("(b t) d -> b t d", b=Bl) for s in scans}
    houtv = {s: hout[s].ap().rearrange("(b t) d -> b t d", b=Bl) for s in scans}
    H2 = 2 * H
    strip = {s: _STRIP[i] for i, s in enumerate(scans)}

    def sl(s):
        return slice(strip[s], strip[s] + 16)

    with tc.tile_pool(name="sw", bufs=1) as wpool, \
         tc.tile_pool(name="sgx", bufs=4) as gxpool, \
         tc.tile_pool(name="sst", bufs=1) as stpool, \
         tc.tile_pool(name="sps", bufs=4, space="PSUM") as pspool, \
         tc.tile_pool(name="swk", bufs=3) as wkpool, \
         tc.tile_pool(name="shT", bufs=3) as htpool, \
         tc.tile_pool(name="srng", bufs=3) as rpool:
        wsb, hT = {}, {}
        for s in scans:
            wsb[s] = wpool.tile([128, KH, G], BF16, tag=f"whh{s}", name=f"whh_{s}")
            for k in range(KH):
                nc.gpsimd.dma_start(wsb[s][:, k, :], whh[s][128 * k:128 * (k + 1), :])
            hT[s] = htpool.tile([128, KH * Bl], BF16, tag="hT", name="hT0")
            nc.vector.memset(hT[s][:], 0.0)
        c_all = stpool.tile([48, H], F32, tag="c", name="c_all")
        nc.vector.memset(c_all[:], 0.0)
        gxc = {}
        gA = [None]
        gB = [None]
        ring = [None]
        nwin = (Tn + TC - 1) // TC

        def load_gx(w):
            # prefetch gx window w (steps w*TC .. w*TC+TC-1)
            tl = gxpool.tile([48, TC, G], BF16, tag="gx", name="gxc")
            for s in scans:
                nc.gpsimd.dma_start(tl[sl(s), :, :],
                                    gxv[s][:, w * TC:(w + 1) * TC, :])
            gxc[w] = tl
            gxc.pop(w - 3, None)

        def inject(tt):
            # psum halves for step tt, pre-filled with gx via identity matmuls
            gA[0] = pspool.tile([48, H2], F32, tag="ps", name="gA")
            gB[0] = pspool.tile([48, H2], F32, tag="ps", name="gB")
            gxt = gxc[tt // TC]
            for half, lo in ((gA[0], 0), (gB[0], H2)):
                for n in range(2):
                    for s in scans:
                        p = strip[s]
                        nc.tensor.matmul(
                            half[sl(s), 512 * n:512 * (n + 1)], id_all[sl(s), :],
                            gxt[sl(s), tt % TC, lo + 512 * n:lo + 512 * (n + 1)],
                            start=True, stop=False, skip_group_check=True,
                            tile_position=(p, p))

        load_gx(0)
        if nwin > 1:
            load_gx(1)
        inject(0)
        for t in range(Tn):
            # recurrent matmuls accumulate onto the injected gx; ScalarE
            # reads gates from PSUM as each half-group completes
            gAc, gBc = gA[0], gB[0]
            for half, cols in ((gAc, (0, 1)), (gBc, (3, 2))):
                for k in range(KH):
                    for n in cols:
                        dst_lo = 512 * (n % 2)
                        for s in scans:
                            nc.tensor.matmul(
                                half[sl(s), dst_lo:dst_lo + 512],
                                hT[s][:, Bl * k:Bl * (k + 1)],
                                wsb[s][:, k, 512 * n:512 * (n + 1)],
                                start=False, stop=(k == KH - 1),
                                skip_group_check=True,
                                tile_position=(0, strip[s]))
            gact = wkpool.tile([48, G], F32, tag="gact", name="gact")
            if t % RC == 0:
                ring[0] = rpool.tile([48, RC, H], BF16, tag="ring", name="ring")
            for s in scans:
                nc.scalar.activation(gact[sl(s), 0:H2], gAc[sl(s), :], AF.Sigmoid)
            for s in scans:
                nc.scalar.activation(gact[sl(s), 3 * H:G], gBc[sl(s), H:H2], AF.Tanh)
            for s in scans:
                nc.scalar.activation(gact[sl(s), H2:3 * H], gBc[sl(s), 0:H], AF.Sigmoid)
            t1 = wkpool.tile([48, H], F32, tag="t1", name="t1")
            t2 = wkpool.tile([48, H], F32, tag="t2", name="t2")
            for s in scans:
                nc.vector.tensor_tensor(out=t1[sl(s), :], in0=gact[sl(s), H:H2],
                                        in1=c_all[sl(s), :], op=ALU.mult)
            for s in scans:
                nc.vector.tensor_tensor(out=t2[sl(s), :], in0=gact[sl(s), 0:H],
                                        in1=gact[sl(s), 3 * H:G], op=ALU.mult)
            for s in scans:
                nc.vector.tensor_tensor(out=c_all[sl(s), :], in0=t1[sl(s), :],
                                        in1=t2[sl(s), :], op=ALU.add)
            tch = wkpool.tile([48, H], F32, tag="tch", name="tch")
            for s in scans:
                nc.scalar.activation(tch[sl(s), :], c_all[sl(s), :], AF.Tanh)
            h16 = wkpool.tile([48, H], BF16, tag="h16", name="h16")
            for s in scans:
                nc.vector.tensor_tensor(out=h16[sl(s), :], in0=gact[sl(s), H2:3 * H],
                                        in1=tch[sl(s), :], op=ALU.mult)
            # allocate transpose psum tiles first (keeps the slot rotation),
            # then emit next step's gx injects BEFORE the transpose matmuls
            # so they fill the PE window spent waiting for h16
            hT_ps = {}
            for s in scans:
                hT_ps[s] = pspool.tile([128, KH * Bl], F32, tag="ps", name="hT_ps")
            if t + 1 < Tn:
                if (t + 1) % TC == 0 and (t + 1) // TC + 1 < nwin:
                    load_gx((t + 1) // TC + 1)
                inject(t + 1)
            for s in scans:
                for k in range(KH):
                    nc.tensor.matmul(hT_ps[s][:, Bl * k:Bl * (k + 1)],
                                     h16[sl(s), 128 * k:128 * (k + 1)],
                                     id_all[sl(s), :], start=True, stop=True,
                                     tile_position=(strip[s], 0))
                hTn = htpool.tile([128, KH * Bl], BF16, tag="hT", name="hTn")
                nc.scalar.activation(hTn[:], hT_ps[s][:], AF.Copy)
                hT[s] = hTn
            for s in scans:
                nc.vector.tensor_scalar_mul(ring[0][sl(s), t % RC, :], h16[sl(s), :],
                                            mask_all[sl(s), t:t + 1])
            if (t + 1) % RC == 0:
                t0r = t + 1 - RC
                for s in scans:
                    nc.gpsimd.dma_start(houtv[s][:, t0r:t0r + RC, :],
                                        ring[0][sl(s), :, :])


def _prep_inputs(inputs, Tn=T, Bl=BL, ncores=NC):
    """Host-side sharding + weight preprocessing. Returns per-core in_maps."""
    x = np.asarray(inputs["x"]).astype(np.int32)
    lengths = np.asarray(inputs["lengths"]).astype(np.int32)
    emb = np.asarray(inputs["emb"], dtype=np.float32)
    ntok = Bl * Tn

    com = {"emb": emb, "ident": np.eye(16, dtype=ml_dtypes.bfloat16)}
    for s in ("f1", "b1", "f2", "b2"):
        w_ih = np.asarray(inputs[f"W_ih_{s}"], np.float32)[_GATE_PERM]
        w_hh = np.asarray(inputs[f"W_hh_{s}"], np.float32)[_GATE_PERM]
        b = np.asarray(inputs[f"b_{s}"], np.float32)[_GATE_PERM]
        com[f"wihT_{s}"] = np.ascontiguousarray(w_ih.T).astype(ml_dtypes.bfloat16)
        com[f"whhT_{s}"] = np.ascontiguousarray(w_hh.T).astype(ml_dtypes.bfloat16)
        com[f"bias_{s}"] = np.tile(b.reshape(1, G), (128, 1))
    com["wclsT"] = np.ascontiguousarray(
        np.asarray(inputs["W_cls"], np.float32).T).astype(ml_dtypes.bfloat16)
    com["bcls"] = np.asarray(inputs["b_cls"], np.float32).reshape(TAGS, 1)

    def chunked(a):  # [ntok] -> [128, ntok//128] with chunk c in column c
        return np.ascontiguousarray(a.reshape(-1).reshape(ntok // 128, 128).T)

    in_maps = []
    for c in range(ncores):
        xs = x[Bl * c:Bl * (c + 1), :Tn]
        ls = np.minimum(lengths[Bl * c:Bl * (c + 1)], Tn)
        ts = np.arange(Tn)[None, :]
        rev = np.where(ts < ls[:, None], ls[:, None] - 1 - ts, ts)  # [Bl,Tn]
        xrev = np.take_along_axis(xs, rev, axis=1)
        flat_rev = (np.arange(Bl)[:, None] * Tn + rev).astype(np.int32)
        m = {
            "xf_idx": chunked(xs),
            "xb_idx": chunked(xrev),
            "rev128": chunked(flat_rev),
            "mask": (ts < ls[:, None]).astype(np.float32),
        }
        m.update(com)
        in_maps.append(m)
    return in_maps


_CACHED = {}


def kernel(**inputs) -> np.ndarray:
    if "nc" not in _CACHED:
        nc = bacc.Bacc("TRN2", target_bir_lowering=False, debug=False,
                       num_devices=NC)
        _build(nc)
        nc.compile()
        _CACHED["nc"] = nc
    nc = _CACHED["nc"]
    in_maps = _prep_inputs(inputs)
    res = run_bass_kernel_spmd(nc, in_maps, core_ids=list(range(NC)), trace=False)
    outs = []
    for c in range(NC):
        lt = res.results[c]["logitsT"]  # [TAGS, ntok]
        outs.append(np.ascontiguousarray(lt.T.reshape(BL, T, TAGS)))
    return np.concatenate(outs, axis=0).astype(np.float32)


# revision 19
# speedup vs baseline: 1.2005x; 1.0298x over previous
"""BiLSTM tagger kernel, direction-parallel sharding over 8 NeuronCores.

Core c in 0..3 runs the FORWARD direction for sequences [32c, 32c+32);
core c+4 runs the BACKWARD direction for the same sequences (inputs
pre-reversed per sequence length on the host). Every matmul therefore has
M=32 batch rows instead of 16 — recurrent matmul cost is N-bound, so this
halves per-core PE work per step — and each core runs ONE scan per layer,
halving ScalarE/VectorE contention.

Between layers, the forward/backward halves are exchanged with a pairwise
AllGather (replica groups {c, c+4}) of the layer's scan-order output into
hpair [2*ntok, H]: slot 0 = lower rank = forward data on BOTH cores, so
the program is identical across cores; all per-core asymmetry (reversal
indices, which weights, which token half of the classifier) lives in
host-computed input tensors. Gate order i,f,o,g (sigmoid block first),
all matmuls bf16, PSUM fp32, gx injected into PSUM via identity matmuls
one step ahead.
"""

import sys

for _p in ("/opt/trn_rl_repo",):
    if _p not in sys.path:
        sys.path.append(_p)

import numpy as np
import ml_dtypes

import concourse.bass as bass
import concourse.tile as tile
from concourse import bacc, mybir
from concourse.bass import IndirectOffsetOnAxis
from concourse.bass_utils import run_bass_kernel_spmd

F32 = mybir.dt.float32
BF16 = mybir.dt.bfloat16
I32 = mybir.dt.int32
AF = mybir.ActivationFunctionType
ALU = mybir.AluOpType

B, T, V, E, H, TAGS = 128, 512, 50000, 256, 512, 64
NC = 8
NPAIR = NC // 2          # 4 sequence groups
BL = B // NPAIR          # 32 sequences per core (one direction each)
G = 4 * H
GROUPS = [[c, c + NPAIR] for c in range(NPAIR)]

_GATE_PERM = np.concatenate([
    np.arange(0, H), np.arange(H, 2 * H), np.arange(3 * H, 4 * H),
    np.arange(2 * H, 3 * H)])


def _build(nc, Tn=T, Bl=BL, TC=2, RC=4):
    ntok = Bl * Tn            # 16384 per core
    nchunk = ntok // 128      # 128
    ncls = ntok // 2 // 128   # 64 classifier chunks (half the pair's tokens)
    KE = E // 128
    KH2 = 2 * H // 128
    KH = H // 128

    # ---- dram I/O (per-core data resolves fwd/bwd asymmetry) ----
    emb = nc.dram_tensor("emb", [V, E], F32, kind="ExternalInput")
    xg_idx = nc.dram_tensor("xg_idx", [128, nchunk], I32, kind="ExternalInput")
    iA2 = nc.dram_tensor("iA2", [128, nchunk], I32, kind="ExternalInput")
    iB2 = nc.dram_tensor("iB2", [128, nchunk], I32, kind="ExternalInput")
    icA = nc.dram_tensor("icA", [128, ncls], I32, kind="ExternalInput")
    icB = nc.dram_tensor("icB", [128, ncls], I32, kind="ExternalInput")
    mask = nc.dram_tensor("mask", [Bl, Tn], F32, kind="ExternalInput")
    ident = nc.dram_tensor("ident", [32, 32], BF16, kind="ExternalInput")

    wih, whh, biasd = {}, {}, {}
    for l, din in (("l1", E), ("l2", 2 * H)):
        wih[l] = nc.dram_tensor(f"wihT_{l}", [din, G], BF16, kind="ExternalInput")
        whh[l] = nc.dram_tensor(f"whhT_{l}", [H, G], BF16, kind="ExternalInput")
        biasd[l] = nc.dram_tensor(f"bias_{l}", [128, G], F32, kind="ExternalInput")
    wcls = nc.dram_tensor("wclsT", [2 * H, TAGS], BF16, kind="ExternalInput")
    bcls = nc.dram_tensor("bcls", [TAGS, 1], F32, kind="ExternalInput")

    gx = {l: nc.dram_tensor(f"gx_{l}", [ntok, G], BF16) for l in ("l1", "l2")}
    hloc = {l: nc.dram_tensor(f"hloc_{l}", [ntok, H], BF16) for l in ("l1", "l2")}
    hpair = {l: nc.dram_tensor(f"hpair_{l}", [2 * ntok, H], BF16)
             for l in ("l1", "l2")}
    logitsT = nc.dram_tensor("logitsT", [TAGS, ntok // 2], F32,
                             kind="ExternalOutput")

    with tile.TileContext(nc) as tc:
        with tc.tile_pool(name="const", bufs=1) as cpool:
            def load_const(nm, shape, dt, src_ap):
                t = cpool.tile(shape, dt, name=nm, tag=nm)
                nc.gpsimd.dma_start(t[:], src_ap)
                return t

            xg_sb = load_const("xg_sb", [128, nchunk], I32, xg_idx[:])
            iA2_sb = load_const("iA2_sb", [128, nchunk], I32, iA2[:])
            iB2_sb = load_const("iB2_sb", [128, nchunk], I32, iB2[:])
            icA_sb = load_const("icA_sb", [128, ncls], I32, icA[:])
            icB_sb = load_const("icB_sb", [128, ncls], I32, icB[:])
            mask_sb = load_const("mask_sb", [Bl, Tn], F32, mask[:])
            id_sb = load_const("id_sb", [32, 32], BF16, ident[:])
            bcls_sb = load_const("bcls_sb", [TAGS, 1], F32, bcls[:])
            bias_sb = {l: load_const(f"bias_sb_{l}", [128, G], F32, biasd[l][:])
                       for l in ("l1", "l2")}
            wcls_sb = cpool.tile([128, KH2, TAGS], BF16, name="wcls_sb")
            for k in range(KH2):
                nc.gpsimd.dma_start(wcls_sb[:, k, :], wcls[128 * k:128 * (k + 1), :])

            # layer-1 proj (emb gather) + scan + exchange
            _proj(nc, tc, nchunk, KE, wih["l1"], bias_sb["l1"], gx["l1"],
                  emb, xg_sb, None, None, is_emb=True)
            _scan(nc, tc, Tn, Bl, TC, RC, KH, whh["l1"], gx["l1"], hloc["l1"],
                  mask_sb, id_sb)
            nc.gpsimd.collective_compute(
                "AllGather", ALU.bypass, GROUPS,
                ins=[hloc["l1"][:]], outs=[hpair["l1"][:]])
            # layer-2 proj (gathers from hpair) + scan + exchange
            _proj(nc, tc, nchunk, KH2, wih["l2"], bias_sb["l2"], gx["l2"],
                  hpair["l1"], None, iA2_sb, iB2_sb, is_emb=False)
            _scan(nc, tc, Tn, Bl, TC, RC, KH, whh["l2"], gx["l2"], hloc["l2"],
                  mask_sb, id_sb)
            nc.gpsimd.collective_compute(
                "AllGather", ALU.bypass, GROUPS,
                ins=[hloc["l2"][:]], outs=[hpair["l2"][:]])

            # classifier over this core's half of the pair's tokens
            with tc.tile_pool(name="cls", bufs=3) as gp, \
                 tc.tile_pool(name="clsT", bufs=3) as gtp, \
                 tc.tile_pool(name="clsps", bufs=4, space="PSUM") as pp, \
                 tc.tile_pool(name="clso", bufs=3) as op:
                for c in range(ncls):
                    o2 = gp.tile([128, 2 * H], BF16, tag="in")
                    nc.gpsimd.indirect_dma_start(
                        out=o2[:, 0:H], out_offset=None, in_=hpair["l2"][:],
                        in_offset=IndirectOffsetOnAxis(ap=icA_sb[:, c:c + 1], axis=0))
                    nc.gpsimd.indirect_dma_start(
                        out=o2[:, H:2 * H], out_offset=None, in_=hpair["l2"][:],
                        in_offset=IndirectOffsetOnAxis(ap=icB_sb[:, c:c + 1], axis=0))
                    o2T = gtp.tile([128, KH2, 128], BF16, tag="inT")
                    for k in range(KH2):
                        nc.sync.dma_start_transpose(
                            o2T[:, k, :], o2[:, 128 * k:128 * (k + 1)])
                    ps = pp.tile([TAGS, 128], F32, name="clsps_t")
                    for k in range(KH2):
                        nc.tensor.matmul(ps[:], wcls_sb[:, k, :], o2T[:, k, :],
                                         start=(k == 0), stop=(k == KH2 - 1))
                    lg = op.tile([TAGS, 128], F32, tag="lg")
                    nc.scalar.activation(lg[:], ps[:], AF.Identity,
                                         bias=bcls_sb[:, 0:1])
                    nc.gpsimd.dma_start(logitsT[:, 128 * c:128 * (c + 1)], lg[:])
    return nc


def _proj(nc, tc, nchunk, KD, wih_d, bias_t, gx_d, src, emb_idx, iA, iB, is_emb):
    """gx = input @ W_ih^T + b in scan-time order. Layer 1: fp32 emb row
    gather + cast. Layer 2: two bf16 row gathers from hpair."""
    D = KD * 128
    G_ = G
    with tc.tile_pool(name="pw", bufs=1) as wpool, \
         tc.tile_pool(name="pg", bufs=3) as gpool, \
         tc.tile_pool(name="pgT", bufs=3) as tpool, \
         tc.tile_pool(name="pps", bufs=4, space="PSUM") as ppool, \
         tc.tile_pool(name="pout", bufs=3) as opool:
        wsb = wpool.tile([128, KD, G_], BF16, tag="w", name="wih_sb")
        for k in range(KD):
            nc.gpsimd.dma_start(wsb[:, k, :], wih_d[128 * k:128 * (k + 1), :])
        for c in range(nchunk):
            if is_emb:
                e32 = gpool.tile([128, D], F32, tag="e32")
                nc.gpsimd.indirect_dma_start(
                    out=e32[:], out_offset=None, in_=src[:],
                    in_offset=IndirectOffsetOnAxis(ap=emb_idx[:, c:c + 1], axis=0))
                xin = gpool.tile([128, D], BF16, tag="e16")
                nc.vector.tensor_copy(xin[:], e32[:])
            else:
                xin = gpool.tile([128, D], BF16, tag="e16")
                nc.gpsimd.indirect_dma_start(
                    out=xin[:, 0:H], out_offset=None, in_=src[:],
                    in_offset=IndirectOffsetOnAxis(ap=iA[:, c:c + 1], axis=0))
                nc.gpsimd.indirect_dma_start(
                    out=xin[:, H:2 * H], out_offset=None, in_=src[:],
                    in_offset=IndirectOffsetOnAxis(ap=iB[:, c:c + 1], axis=0))
            xT = tpool.tile([128, KD, 128], BF16, tag="xT")
            for k in range(KD):
                nc.sync.dma_start_transpose(xT[:, k, :], xin[:, 128 * k:128 * (k + 1)])
            gout = opool.tile([128, G_], BF16, tag="gout")
            for n in range(G_ // 512):
                ps = ppool.tile([128, 512], F32, name="pps")
                for k in range(KD):
                    nc.tensor.matmul(
                        ps[:], xT[:, k, :], wsb[:, k, 512 * n:512 * (n + 1)],
                        start=(k == 0), stop=(k == KD - 1))
                nc.vector.tensor_tensor(
                    out=gout[:, 512 * n:512 * (n + 1)], in0=ps[:],
                    in1=bias_t[:, 512 * n:512 * (n + 1)], op=ALU.add)
            nc.gpsimd.dma_start(gx_d[128 * c:128 * (c + 1), :], gout[:])


def _scan(nc, tc, Tn, Bl, TC, RC, KH, whh_d, gx_d, hout_d, mask_sb, id_sb):
    """Single-direction scan, M=32 batch. Same software-pipelined gx
    injection as the 2-scan variant, one scan per core."""
    gxv = gx_d.ap().rearrange("(b t) d -> b t d", b=Bl)
    houtv = hout_d.ap().rearrange("(b t) d -> b t d", b=Bl)
    H2 = 2 * H
    with tc.tile_pool(name="sw", bufs=1) as wpool, \
         tc.tile_pool(name="sgx", bufs=4) as gxpool, \
         tc.tile_pool(name="sst", bufs=1) as stpool, \
         tc.tile_pool(name="sps", bufs=4, space="PSUM") as pspool, \
         tc.tile_pool(name="swk", bufs=3) as wkpool, \
         tc.tile_pool(name="shT", bufs=3) as htpool, \
         tc.tile_pool(name="srng", bufs=3) as rpool:
        wsb = wpool.tile([128, KH, G], BF16, tag="whh", name="whh_sb")
        for k in range(KH):
            nc.gpsimd.dma_start(wsb[:, k, :], whh_d[128 * k:128 * (k + 1), :])
        hT = [htpool.tile([128, KH * Bl], BF16, tag="hT", name="hT0")]
        nc.vector.memset(hT[0][:], 0.0)
        c_st = stpool.tile([Bl, H], F32, tag="c", name="c_st")
        nc.vector.memset(c_st[:], 0.0)
        gxc = {}
        gA = [None]
        gB = [None]
        ring = [None]
        nwin = (Tn + TC - 1) // TC

        def load_gx(w):
            tl = gxpool.tile([Bl, TC, G], BF16, tag="gx", name="gxc")
            nc.gpsimd.dma_start(tl[:], gxv[:, w * TC:(w + 1) * TC, :])
            gxc[w] = tl
            gxc.pop(w - 3, None)

        def inject(tt):
            gA[0] = pspool.tile([Bl, H2], F32, tag="ps", name="gA")
            gB[0] = pspool.tile([Bl, H2], F32, tag="ps", name="gB")
            gxt = gxc[tt // TC]
            for half, lo in ((gA[0], 0), (gB[0], H2)):
                for n in range(2):
                    nc.tensor.matmul(
                        half[:, 512 * n:512 * (n + 1)], id_sb[:],
                        gxt[:, tt % TC, lo + 512 * n:lo + 512 * (n + 1)],
                        start=True, stop=False, skip_group_check=True)

        load_gx(0)
        if nwin > 1:
            load_gx(1)
        inject(0)
        for t in range(Tn):
            gAc, gBc = gA[0], gB[0]
            for half, cols in ((gAc, (0, 1)), (gBc, (3, 2))):
                for n in cols:
                    dst_lo = 512 * (n % 2)
                    for k in range(KH):
                        nc.tensor.matmul(
                            half[:, dst_lo:dst_lo + 512],
                            hT[0][:, Bl * k:Bl * (k + 1)],
                            wsb[:, k, 512 * n:512 * (n + 1)],
                            start=False, stop=(k == KH - 1),
                            skip_group_check=True)
            gact = wkpool.tile([Bl, G], F32, tag="gact", name="gact")
            if t % RC == 0:
                ring[0] = rpool.tile([Bl, RC, H], BF16, tag="ring", name="ring")
            nc.scalar.activation(gact[:, 0:H2], gAc[:], AF.Sigmoid)
            nc.scalar.activation(gact[:, 3 * H:G], gBc[:, H:H2], AF.Tanh)
            nc.scalar.activation(gact[:, H2:3 * H], gBc[:, 0:H], AF.Sigmoid)
            t1 = wkpool.tile([Bl, H], F32, tag="t1", name="t1")
            t2 = wkpool.tile([Bl, H], F32, tag="t2", name="t2")
            nc.vector.tensor_tensor(out=t1[:], in0=gact[:, H:H2], in1=c_st[:],
                                    op=ALU.mult)
            nc.vector.tensor_tensor(out=t2[:], in0=gact[:, 0:H],
                                    in1=gact[:, 3 * H:G], op=ALU.mult)
            nc.vector.tensor_tensor(out=c_st[:], in0=t1[:], in1=t2[:], op=ALU.add)
            tch = wkpool.tile([Bl, H], F32, tag="tch", name="tch")
            nc.scalar.activation(tch[:], c_st[:], AF.Tanh)
            h16 = wkpool.tile([Bl, H], BF16, tag="h16", name="h16")
            nc.vector.tensor_tensor(out=h16[:], in0=gact[:, H2:3 * H],
                                    in1=tch[:], op=ALU.mult)
            hT_ps = pspool.tile([128, KH * Bl], F32, tag="ps", name="hT_ps")
            if t + 1 < Tn:
                if (t + 1) % TC == 0 and (t + 1) // TC + 1 < nwin:
                    load_gx((t + 1) // TC + 1)
                inject(t + 1)
            for k in range(KH):
                nc.tensor.matmul(hT_ps[:, Bl * k:Bl * (k + 1)],
                                 h16[:, 128 * k:128 * (k + 1)], id_sb[:],
                                 start=True, stop=True)
            hTn = htpool.tile([128, KH * Bl], BF16, tag="hT", name="hTn")
            nc.scalar.activation(hTn[:], hT_ps[:], AF.Copy)
            hT[0] = hTn
            nc.vector.tensor_scalar_mul(ring[0][:, t % RC, :], h16[:],
                                        mask_sb[:, t:t + 1])
            if (t + 1) % RC == 0:
                t0r = t + 1 - RC
                nc.gpsimd.dma_start(houtv[:, t0r:t0r + RC, :], ring[0][:, :, :])


def _prep_inputs(inputs, Tn=T, Bl=BL):
    x = np.asarray(inputs["x"]).astype(np.int32)
    lengths = np.asarray(inputs["lengths"]).astype(np.int32)
    emb = np.asarray(inputs["emb"], dtype=np.float32)
    ntok = Bl * Tn
    bf = ml_dtypes.bfloat16

    wt = {}
    for s in ("f1", "b1", "f2", "b2"):
        w_ih = np.asarray(inputs[f"W_ih_{s}"], np.float32)[_GATE_PERM]
        w_hh = np.asarray(inputs[f"W_hh_{s}"], np.float32)[_GATE_PERM]
        b = np.asarray(inputs[f"b_{s}"], np.float32)[_GATE_PERM]
        wt[f"wihT_{s}"] = np.ascontiguousarray(w_ih.T).astype(bf)
        wt[f"whhT_{s}"] = np.ascontiguousarray(w_hh.T).astype(bf)
        wt[f"bias_{s}"] = np.tile(b.reshape(1, G), (128, 1))
    com = {"emb": emb, "ident": np.eye(32, dtype=bf),
           "wclsT": np.ascontiguousarray(
               np.asarray(inputs["W_cls"], np.float32).T).astype(bf),
           "bcls": np.asarray(inputs["b_cls"], np.float32).reshape(TAGS, 1)}

    def chunked(a):
        return np.ascontiguousarray(a.reshape(-1).reshape(-1, 128).T)

    in_maps = [None] * NC
    for p in range(NPAIR):
        xs = x[Bl * p:Bl * (p + 1), :Tn]
        ls = np.minimum(lengths[Bl * p:Bl * (p + 1)], Tn)
        ts = np.arange(Tn)[None, :]
        rev = np.where(ts < ls[:, None], ls[:, None] - 1 - ts, ts)  # [Bl,Tn]
        base = np.arange(Bl)[:, None] * Tn + ts                      # natural
        base_rev = np.arange(Bl)[:, None] * Tn + rev                 # reversed
        m_common = {"mask": (ts < ls[:, None]).astype(np.float32)}
        m_common.update(com)

        # classifier token halves: fwd core -> seqs [0:Bl//2), bwd -> rest
        def cls_idx(b0, slotA_rev):
            tok = (np.arange(b0 * Tn, (b0 + Bl // 2) * Tn))
            bb, tt2 = tok // Tn, tok % Tn
            iA_ = bb * Tn + tt2
            iB_ = ntok + bb * Tn + rev[bb, tt2]
            if slotA_rev:
                pass
            return chunked(iA_.astype(np.int32)), chunked(iB_.astype(np.int32))

        for half, core in ((0, p), (1, p + NPAIR)):
            if half == 0:   # forward core
                m = {"xg_idx": chunked(xs),
                     "iA2": chunked(base.astype(np.int32)),
                     "iB2": chunked((ntok + base_rev).astype(np.int32)),
                     "wihT_l1": wt["wihT_f1"], "whhT_l1": wt["whhT_f1"],
                     "bias_l1": wt["bias_f1"],
                     "wihT_l2": wt["wihT_f2"], "whhT_l2": wt["whhT_f2"],
                     "bias_l2": wt["bias_f2"]}
                iA_c, iB_c = cls_idx(0, False)
            else:           # backward core
                xrev = np.take_along_axis(xs, rev, axis=1)
                m = {"xg_idx": chunked(xrev),
                     "iA2": chunked(base_rev.astype(np.int32)),
                     "iB2": chunked((ntok + base).astype(np.int32)),
                     "wihT_l1": wt["wihT_b1"], "whhT_l1": wt["whhT_b1"],
                     "bias_l1": wt["bias_b1"],
                     "wihT_l2": wt["wihT_b2"], "whhT_l2": wt["whhT_b2"],
                     "bias_l2": wt["bias_b2"]}
                iA_c, iB_c = cls_idx(Bl // 2, False)
            m["icA"], m["icB"] = iA_c, iB_c
            m.update(m_common)
            in_maps[core] = m
    return in_maps


_CACHED = {}


def kernel(**inputs) -> np.ndarray:
    if "nc" not in _CACHED:
        nc = bacc.Bacc("TRN2", target_bir_lowering=False, debug=False,
                       num_devices=NC)
        _build(nc)
        nc.compile()
        _CACHED["nc"] = nc
    nc = _CACHED["nc"]
    in_maps = _prep_inputs(inputs)
    res = run_bass_kernel_spmd(nc, in_maps, core_ids=list(range(NC)), trace=False)
    out = np.empty((B, T, TAGS), np.float32)
    for p in range(NPAIR):
        for half, core in ((0, p), (1, p + NPAIR)):
            lt = res.results[core]["logitsT"]          # [TAGS, ntok//2]
            seqs = lt.T.reshape(BL // 2, T, TAGS)
            b0 = BL * p + half * (BL // 2)
            out[b0:b0 + BL // 2] = seqs
    return out.astype(np.float32)
